# revision 8
# baseline (speedup 1.0000x reference)
"""Trainium2 Bass kernel for CustomGRU (B=64, T=512, D=512, U=1024).

Sharding: data-parallel over batch across 8 NeuronCores (8 rows each),
weights replicated. Per core:

  Phase 1 (projections): xzr[t,b,:] = X[b,t,:] @ [Wz|Wr|Wh] + b  -> DRAM
    - stationary = X^T tiles (host-pre-transposed), moving = Wcat.
  Phase 2 (recurrence), per step t:
    - gate pre-activations via PSUM accumulation: an "eye-matmul" preloads
      xzr_t into PSUM (out = eye8.T @ xzr_t), then 8 K-chunk matmuls
      accumulate h_{t-1} @ U on top. Stationary = h^T chunks [128,8]
      (cheap LDWEIGHTS), moving = U weight slices [128,512] (1 cyc/row
      with float32r since N=512 >= 256).
    - sigmoid/tanh on ScalarE evict PSUM -> SBUF (B-major [8,1024]).
    - r is PE-transposed to U-major [128,64] to form (r*h)^T, the
      stationary operand of the candidate matmul.
    - combine in B-major on VectorE: h = hh + z*(h_prev - hh).
    - h_new is PE-transposed back to h^T for the next step's stationary.

All matmuls run in float32r (fp32 storage, TF32-like PE mode, full rate);
everything else is fp32.
"""
import sys

if "/opt/trn_rl_repo" not in sys.path:
    sys.path.insert(0, "/opt/trn_rl_repo")

import numpy as np
from contextlib import ExitStack

import concourse.bass as bass
import concourse.bacc as bacc
import concourse.tile as tile
from concourse import mybir
from concourse.bass_utils import run_bass_kernel_spmd

F32 = mybir.dt.float32
F32R = mybir.dt.float32r

N_CORES = 8
B = 64
BS = B // N_CORES  # 8 batch rows per core
D = 512
U = 1024
U3 = 3 * U        # 3072 (z|r|h)
KC = U // 128     # 8 contraction chunks of 128
DC = D // 128     # 4 input-dim chunks


def r32(ap):
    return ap.bitcast(F32R)


def build(nc, T):
    BT = BS * T

    # ---- DRAM I/O (per-core) ----
    xT_d = nc.dram_tensor("xT", [D, BT], F32R, kind="ExternalInput")
    wcat_d = nc.dram_tensor("wcat", [D, U3], F32R, kind="ExternalInput")
    bb_d = nc.dram_tensor("bb", [128, U3], F32, kind="ExternalInput")
    uzr_d = nc.dram_tensor("uzr", [U, 2 * U], F32R, kind="ExternalInput")
    uh_d = nc.dram_tensor("uh", [U, U], F32R, kind="ExternalInput")
    eye8r_d = nc.dram_tensor("eye8r", [BS, BS], F32R, kind="ExternalInput")
    eye8f_d = nc.dram_tensor("eye8f", [BS, BS], F32, kind="ExternalInput")
    out_d = nc.dram_tensor("out", [T, BS, U], F32, kind="ExternalOutput")

    with tile.TileContext(nc) as tc, ExitStack() as ctx:
        dram = ctx.enter_context(tc.tile_pool(name="dram", bufs=1, space="DRAM"))
        xzr_d = dram.tile([T, BS, U3], F32R)

        const = ctx.enter_context(tc.tile_pool(name="const", bufs=1))
        eye8r = const.tile([BS, BS], F32R)
        nc.sync.dma_start(eye8r[:], eye8r_d[:])
        eye8f = const.tile([BS, BS], F32)
        nc.sync.dma_start(eye8f[:], eye8f_d[:])

        # ---------------- Phase 1: input projections ----------------
        with ExitStack() as p1:
            wpool = p1.enter_context(tc.tile_pool(name="wcat", bufs=1))
            wcat = wpool.tile([128, DC * U3], F32R)  # [p, dc, u]
            nc.sync.dma_start(
                wcat[:].rearrange("p (dc u) -> p dc u", dc=DC),
                wcat_d.rearrange("(dc p) u -> p dc u", p=128),
            )
            bb = wpool.tile([128, U3], F32)
            nc.sync.dma_start(bb[:], bb_d[:])

            xp = p1.enter_context(tc.tile_pool(name="xT", bufs=3))
            op = p1.enter_context(tc.tile_pool(name="p1out", bufs=3))
            pp = p1.enter_context(tc.tile_pool(name="p1ps", bufs=4, space="PSUM"))

            n_bt = BT // 128          # bt-chunks of 128 (4 per batch row)
            tpb = T // 128            # t-chunks per batch row
            for tb in range(n_bt):
                b_idx, t_blk = tb // tpb, tb % tpb
                xt = xp.tile([128, DC * 128], F32R, tag="xt")  # [p=d, dc, bt]
                nc.sync.dma_start(
                    xt[:].rearrange("p (dc n) -> p dc n", dc=DC),
                    xT_d[:, tb * 128:(tb + 1) * 128].rearrange(
                        "(dc p) n -> p dc n", p=128
                    ),
                )
                for ut in range(U3 // 512):
                    ps = pp.tile([128, 512], F32, tag="ps")
                    for dc in range(DC):
                        nc.tensor.matmul(
                            ps[:],
                            xt[:, dc * 128:(dc + 1) * 128],
                            wcat[:, dc * U3 + ut * 512: dc * U3 + ut * 512 + 512],
                            start=(dc == 0),
                            stop=(dc == DC - 1),
                        )
                    ob = op.tile([128, 512], F32R, tag="ob")
                    nc.vector.tensor_add(
                        ob[:], ps[:], bb[:, ut * 512:(ut + 1) * 512]
                    )
                    nc.sync.dma_start(
                        xzr_d[
                            t_blk * 128:(t_blk + 1) * 128,
                            b_idx,
                            ut * 512:(ut + 1) * 512,
                        ].squeeze(),
                        ob[:],
                    )

        # ---------------- Phase 2: recurrence ----------------
        upool = ctx.enter_context(tc.tile_pool(name="u", bufs=1))
        uzr = upool.tile([128, KC * 2 * U], F32R)  # [p, k, 2U]
        nc.sync.dma_start(
            uzr[:].rearrange("p (k u) -> p k u", k=KC),
            uzr_d.rearrange("(k p) u -> p k u", p=128),
        )
        uh = upool.tile([128, KC * U], F32R)
        nc.sync.dma_start(
            uh[:].rearrange("p (k u) -> p k u", k=KC),
            uh_d.rearrange("(k p) u -> p k u", p=128),
        )

        hpool = ctx.enter_context(tc.tile_pool(name="h", bufs=2))
        stage = ctx.enter_context(tc.tile_pool(name="stage", bufs=4))
        gates = ctx.enter_context(tc.tile_pool(name="gates", bufs=2))
        psg = ctx.enter_context(tc.tile_pool(name="psg", bufs=4, space="PSUM"))
        pst = ctx.enter_context(tc.tile_pool(name="pst", bufs=2, space="PSUM"))

        hT0 = const.tile([128, KC * BS], F32R)  # h^T chunks: col k*8+b
        nc.any.memzero(hT0[:])
        hT_prev = hT0

        def gate_mms(xoff, uoff, umat, hT, tag):
            """Two [8,512] psum tiles: eye-preload(x) + sum_k h^T_k @ U_k."""
            tiles = []
            for j in range(2):
                ps = psg.tile([BS, 512], F32, tag="psg")
                xz = xz_t[:, xoff + 512 * j: xoff + 512 * j + 512]
                nc.tensor.matmul(ps[:], eye8r[:], xz, start=True, stop=False)
                for k in range(KC):
                    nc.tensor.matmul(
                        ps[:],
                        hT[:, k * BS:(k + 1) * BS],
                        umat[:, k * WSTRIDE + uoff + 512 * j:
                             k * WSTRIDE + uoff + 512 * j + 512],
                        start=False,
                        stop=(k == KC - 1),
                    )
                tiles.append(ps)
            return tiles

        for t in range(T):
            xz_t = stage.tile([BS, U3], F32R, tag="xz")
            nc.sync.dma_start(xz_t[:], xzr_d[t].squeeze())

            # r gate, then transpose to U-major and form (r*h)^T
            WSTRIDE = 2 * U
            ps_r = gate_mms(U, U, uzr, hT_prev, "r")
            r_B = gates.tile([BS, U], F32, tag="r")
            for j in range(2):
                nc.scalar.activation(
                    r_B[:, 512 * j:512 * j + 512], ps_r[j][:],
                    mybir.ActivationFunctionType.Sigmoid,
                )
            # z gate (keeps PE busy while sigmoid(r) runs)
            ps_z = gate_mms(0, 0, uzr, hT_prev, "z")
            z_B = gates.tile([BS, U], F32, tag="z")
            for j in range(2):
                nc.scalar.activation(
                    z_B[:, 512 * j:512 * j + 512], ps_z[j][:],
                    mybir.ActivationFunctionType.Sigmoid,
                )

            rT = pst.tile([128, KC * BS], F32, tag="rT")
            for c in range(KC):
                nc.tensor.transpose(
                    rT[:, c * BS:(c + 1) * BS],
                    r_B[:, c * 128:(c + 1) * 128],
                    eye8f[:],
                )
            rhT = hpool.tile([128, KC * BS], F32R, tag="rhT")
            nc.vector.tensor_mul(rhT[:], rT[:], hT_prev[:])

            # candidate
            WSTRIDE = U
            ps_h = gate_mms(2 * U, 0, uh, rhT, "hh")
            hh_B = gates.tile([BS, U], F32, tag="hh")
            for j in range(2):
                nc.scalar.activation(
                    hh_B[:, 512 * j:512 * j + 512], ps_h[j][:],
                    mybir.ActivationFunctionType.Tanh,
                )

            # combine: h = hh + z * (h_prev - hh)   (B-major, VectorE)
            if t == 0:
                h_B_prev = gates.tile([BS, U], F32, tag="hB")
                nc.any.memzero(h_B_prev[:])
            tmp = gates.tile([BS, U], F32, tag="tmp")
            nc.vector.tensor_sub(tmp[:], h_B_prev[:], hh_B[:])
            nc.vector.tensor_mul(tmp[:], z_B[:], tmp[:])
            h_B = gates.tile([BS, U], F32, tag="hB")
            nc.vector.tensor_add(h_B[:], hh_B[:], tmp[:])

            nc.sync.dma_start(out_d[t].squeeze(), h_B[:])

            # h^T for next step (PE transpose -> PSUM -> SBUF)
            hT_ps = pst.tile([128, KC * BS], F32, tag="hTps")
            for c in range(KC):
                nc.tensor.transpose(
                    hT_ps[:, c * BS:(c + 1) * BS],
                    h_B[:, c * 128:(c + 1) * 128],
                    eye8f[:],
                )
            hT_new = hpool.tile([128, KC * BS], F32R, tag="hT")
            nc.vector.tensor_copy(hT_new[:], hT_ps[:])
            hT_prev = hT_new
            h_B_prev = h_B

    nc.compile()
    return nc


def prepare(inputs, Wz, Uz, bz, Wr, Ur, br, Wh, Uh, bh, T):
    """Build the Bass program and the per-core input maps."""
    x = np.asarray(inputs, dtype=np.float32)[:, :T, :]

    wcat = np.concatenate([Wz, Wr, Wh], axis=1).astype(np.float32)
    bcat = np.concatenate([bz, br, bh]).astype(np.float32)
    bb = np.ascontiguousarray(np.broadcast_to(bcat, (128, U3)))
    uzr = np.concatenate([Uz, Ur], axis=1).astype(np.float32)
    uh = np.asarray(Uh, dtype=np.float32)
    eye8 = np.eye(BS, dtype=np.float32)

    nc = bacc.Bacc("TRN2", target_bir_lowering=False, debug=False,
                   num_devices=N_CORES)
    build(nc, T)

    in_maps = []
    for c in range(N_CORES):
        xc = x[c * BS:(c + 1) * BS]               # [BS, T, D]
        xT = np.ascontiguousarray(xc.reshape(BS * T, D).T)  # [D, BS*T]
        in_maps.append({
            "xT": xT, "wcat": wcat, "bb": bb, "uzr": uzr, "uh": uh,
            "eye8r": eye8, "eye8f": eye8,
        })
    return nc, in_maps


def assemble(results):
    outs = []
    for c in range(N_CORES):
        o = results[c]["out"]                     # [T, BS, U]
        outs.append(np.ascontiguousarray(o.transpose(1, 0, 2)))
    return np.concatenate(outs, axis=0)           # [B, T, U]


def kernel(inputs, Wz, Uz, bz, Wr, Ur, br, Wh, Uh, bh, _T=None):
    T = inputs.shape[1] if _T is None else _T
    nc, in_maps = prepare(inputs, Wz, Uz, bz, Wr, Ur, br, Wh, Uh, bh, T)
    res = run_bass_kernel_spmd(nc, in_maps, list(range(N_CORES)))
    return assemble(res.results)
